# revision 1
# baseline (speedup 1.0000x reference)
"""Causal self-attention (RMS-normed QK, RoPE, GQA) Trainium2 Bass kernel.

Sharding over 8 NeuronCores: 4-way data-parallel over batch x 2-way
tensor-parallel over heads.  Core c handles batch b = c // 2 and head group
g = c % 2 (q heads g*8..g*8+7, kv heads g*2, g*2+1).  Each core produces a
partial output projection; the host sums the two head-group partials per
batch.

Per-core device program (T=2048, D=2048, fp32 data, fp32r matmuls; every
tile consumed by an fp32r matmul is produced with dtype float32r so the
producing instruction emits fp32r-rounded values, as walrus requires):
  P1  x^T via PE transposes; weight-stationary projections producing
      q^T/k^T/v^T [head_dim, t] directly; RMS-norm (partition sums via
      ones-matmul, K=1 broadcast matmul, reciprocal) + gain + RoPE in the
      transposed layout (half-swap via SBUF-SBUF DMA); v^T transposed back
      to natural [t, dv] for PV.
  P2  per head: transposed scores S^T[j,i] = (K^T)^T Q^T, exp (no max
      subtraction needed: |scores| <= gain*sqrt(128)), causal mask via
      affine_select, PV accumulation y^T[dv,i] in PSUM, softmax denominator
      via ones-matmul + K=1 broadcast matmul + reciprocal; normalized y^T
      spilled to DRAM.
  P3  out = y @ out_w^T accumulated over the 8 local heads.
"""

import math

import numpy as np

import concourse.bass as bass
import concourse.mybir as mybir
import concourse.tile as tile
from concourse import bacc, bass_utils
from concourse.masks import make_identity

F32 = mybir.dt.float32
F32R = mybir.dt.float32r

HEAD_DIM = 128
N_HEADS = 16
N_KV_HEADS = 4
ROPE_BASE = 10000.0
TRAIN_SEQ_LEN = 1024

B, D = 4, 2048
H_LOC = 8  # q heads per core
KV_LOC = 2  # kv heads per core
EC = D // 128  # contraction chunks
EPS = float(np.finfo(np.float32).eps)
INV_SQRT_HD = 1.0 / math.sqrt(HEAD_DIM)


def _rope_tables(T):
    rd = HEAD_DIM
    base = ROPE_BASE
    if T > TRAIN_SEQ_LEN:
        scale = T / TRAIN_SEQ_LEN
        base = base * scale ** (rd / (rd - 2))
    inv_freq = 1.0 / base ** (np.arange(0, rd, 2, dtype=np.float32) / rd)
    freqs = np.outer(np.arange(T, dtype=np.float32), inv_freq)
    return np.cos(freqs).astype(np.float32), np.sin(freqs).astype(np.float32)


def build_program(T=2048, phases=(1, 2, 3)):
    """Build the per-core Bass program. T must be a multiple of 512."""
    assert T % 512 == 0
    NT = T // 128  # 128-row t tiles
    NTB = T // 512  # 512-col t blocks
    NIB = T // 512  # attention i blocks

    nc = bacc.Bacc("TRN2", target_bir_lowering=False, debug=False, num_devices=8)

    x_d = nc.dram_tensor("x", [T, D], F32, kind="ExternalInput").ap()
    qwt_d = nc.dram_tensor("qwt", [D, H_LOC * 128], F32R, kind="ExternalInput").ap()
    kwt_d = nc.dram_tensor("kwt", [D, KV_LOC * 128], F32R, kind="ExternalInput").ap()
    vwt_d = nc.dram_tensor("vwt", [D, KV_LOC * 128], F32R, kind="ExternalInput").ap()
    owt_d = nc.dram_tensor("owt", [H_LOC * 128, D], F32R, kind="ExternalInput").ap()
    # cos2 = [cosT; cosT], ssin2 = [sinT; -sinT]  (rope via half-swap DMA)
    cost_d = nc.dram_tensor("cos2", [128, T], F32, kind="ExternalInput").ap()
    sint_d = nc.dram_tensor("ssin2", [128, T], F32, kind="ExternalInput").ap()
    # per-(q|k)-head multipliers: q_gain for the 8 q heads, 1.0 for kv heads
    gains_d = nc.dram_tensor("gains", [H_LOC + KV_LOC], F32, kind="ExternalInput").ap()
    out_d = nc.dram_tensor("out", [T, D], F32, kind="ExternalOutput").ap()

    NH = H_LOC + KV_LOC

    with tile.TileContext(nc) as tc:
        with (
            tc.tile_pool(name="const", bufs=1) as const_p,
            tc.tile_pool(name="dram", bufs=1, space="DRAM") as dram_p,
        ):
            ident = const_p.tile([128, 128], F32)
            make_identity(nc, ident)
            ones_col = const_p.tile([1, 128], F32R)  # lhsT for K=1 bcast matmul
            nc.vector.memset(ones_col.bitcast(F32), 1.0)
            ones_128 = const_p.tile([128, 1], F32R)  # lhsT for partition-sum
            nc.vector.memset(ones_128.bitcast(F32), 1.0)
            eps_sb = const_p.tile([128, 1], F32)
            nc.vector.memset(eps_sb, EPS)
            cost_sb = const_p.tile([128, T], F32)
            nc.sync.dma_start(cost_sb, cost_d)
            sint_sb = const_p.tile([128, T], F32)
            nc.sync.dma_start(sint_sb, sint_d)
            # per-partition-broadcast gains [128, NH]
            gains_sb = const_p.tile([128, NH], F32)
            nc.sync.dma_start(
                gains_sb,
                bass.AP(tensor=gains_d.tensor, offset=gains_d.offset,
                        ap=[[0, 128], *gains_d.ap]),
            )

            ydram = dram_p.tile([H_LOC, 128, T], F32R)

            with (
                tc.tile_pool(name="persist", bufs=1) as pers_p,
            ):
                qT = pers_p.tile([128, H_LOC, T], F32R)
                kT = pers_p.tile([128, KV_LOC, T], F32R)
                v_sb = pers_p.tile([128, NT, KV_LOC * 128], F32R)

                # ---------------- Phase 1: projections -------------------
                with (
                    tc.tile_pool(name="p1x", bufs=2) as x_p,
                    tc.tile_pool(name="p1xt", bufs=1) as xt_p,
                    tc.tile_pool(name="p1w", bufs=2) as w_p,
                    tc.tile_pool(name="p1work", bufs=2) as wk_p,
                    tc.tile_pool(name="p1stat", bufs=3) as st_p,
                    tc.tile_pool(name="p1ps", bufs=2, space="PSUM") as ps_p,
                    tc.tile_pool(name="p1psl", bufs=2, space="PSUM") as ps_l1,
                    tc.tile_pool(name="p1psb", bufs=2, space="PSUM") as ps_b1,
                    tc.tile_pool(name="p1pst", bufs=2, space="PSUM") as ps_t,
                ):
                    for tb in range(NTB):
                        tsl = slice(tb * 512, (tb + 1) * 512)
                        # x^T chunk for this 512-column block
                        xt = xt_p.tile([128, EC, 512], F32R, tag="xt")
                        for tt in range(4):
                            for xh in range(2):  # load x tile in two halves
                                xx = x_p.tile([128, D // 2], F32, tag="x")
                                nc.gpsimd.dma_start(
                                    xx, x_d[(tb * 4 + tt) * 128:
                                            (tb * 4 + tt + 1) * 128,
                                            xh * (D // 2):(xh + 1) * (D // 2)])
                                for ei in range(EC // 2):
                                    ecc = xh * (EC // 2) + ei
                                    pst = ps_t.tile([128, 128], F32, tag="tp")
                                    nc.tensor.transpose(
                                        pst, xx[:, ei * 128:(ei + 1) * 128], ident)
                                    nc.vector.tensor_copy(
                                        xt[:, ecc, tt * 128:(tt + 1) * 128], pst)

                        def load_w(w_dram, col0):
                            wt = w_p.tile([128, EC, 128], F32R, tag="w")
                            nc.gpsimd.dma_start(
                                wt, w_dram[:, col0:col0 + 128]
                                .rearrange("(e p) c -> p e c", p=128))
                            return wt

                        def project(w_dram, col0, rope_dst, gain_idx):
                            """One head's worth: [128, 512] transposed proj +
                            rms + gain + rope, written into rope_dst slice."""
                            wt = load_w(w_dram, col0)
                            h_ps = ps_p.tile([128, 512], F32, tag="proj")
                            for ecc in range(EC):
                                nc.tensor.matmul(
                                    h_ps, wt[:, ecc, :], xt[:, ecc, :],
                                    start=(ecc == 0), stop=(ecc == EC - 1))
                            # sum of squares over head dim (partitions),
                            # broadcast back over partitions, rsqrt
                            sq = wk_p.tile([128, 512], F32R, tag="sq")
                            nc.scalar.activation(
                                sq, h_ps, mybir.ActivationFunctionType.Square)
                            ssq_ps = ps_l1.tile([1, 512], F32, tag="ssq")
                            nc.tensor.matmul(
                                ssq_ps, ones_128, sq, start=True, stop=True)
                            ssq_sb = st_p.tile([1, 512], F32R, tag="ssqs")
                            nc.vector.tensor_copy(ssq_sb, ssq_ps)
                            ssqb_ps = ps_b1.tile([128, 512], F32, tag="ssqb")
                            nc.tensor.matmul(
                                ssqb_ps, ones_col, ssq_sb, start=True, stop=True)
                            rms = wk_p.tile([128, 512], F32, tag="rms")
                            nc.scalar.activation(
                                rms, ssqb_ps, mybir.ActivationFunctionType.Sqrt,
                                bias=eps_sb, scale=1.0 / 128.0)
                            rinv = wk_p.tile([128, 512], F32, tag="rinv")
                            nc.vector.reciprocal(rinv, rms)
                            qg = wk_p.tile([128, 512], F32, tag="qg")
                            nc.vector.tensor_scalar_mul(
                                qg, h_ps, gains_sb[:, gain_idx:gain_idx + 1])
                            qn = wk_p.tile([128, 512], F32, tag="qn")
                            nc.vector.tensor_mul(qn, qg, rinv)
                            # rope: out = qn*cos2 + swap(qn)*ssin2
                            qsw = wk_p.tile([128, 512], F32, tag="qsw")
                            nc.sync.dma_start(qsw[0:64, :], qn[64:128, :])
                            nc.sync.dma_start(qsw[64:128, :], qn[0:64, :])
                            nc.vector.tensor_mul(qsw, qsw, sint_sb[:, tsl])
                            rc = wk_p.tile([128, 512], F32, tag="rc")
                            nc.vector.tensor_mul(rc, qn, cost_sb[:, tsl])
                            nc.vector.tensor_add(rope_dst, rc, qsw)

                        for h in range(H_LOC):
                            project(qwt_d, h * 128, qT[:, h, tsl], h)
                        for kv in range(KV_LOC):
                            project(kwt_d, kv * 128, kT[:, kv, tsl], H_LOC + kv)

                        # v: plain transposed projection, then transpose back
                        for kv in range(KV_LOC):
                            wt = load_w(vwt_d, kv * 128)
                            v_ps = ps_p.tile([128, 512], F32, tag="proj")
                            for ecc in range(EC):
                                nc.tensor.matmul(
                                    v_ps, wt[:, ecc, :], xt[:, ecc, :],
                                    start=(ecc == 0), stop=(ecc == EC - 1))
                            vt = wk_p.tile([128, 512], F32, tag="vt")
                            nc.vector.tensor_copy(vt, v_ps)
                            for tt in range(4):
                                pst = ps_t.tile([128, 128], F32, tag="tp")
                                nc.tensor.transpose(
                                    pst, vt[:, tt * 128:(tt + 1) * 128], ident)
                                nc.vector.tensor_copy(
                                    v_sb[:, tb * 4 + tt,
                                         kv * 128:(kv + 1) * 128], pst)

                # ---------------- Phase 2: attention ---------------------
                with (
                    tc.tile_pool(name="p2pt", bufs=4) as pt_p,
                    tc.tile_pool(name="p2acc", bufs=2) as acc_p,
                    tc.tile_pool(name="p2y", bufs=2) as ystg_p,
                    tc.tile_pool(name="p2r", bufs=2) as r_p,
                    tc.tile_pool(name="p2pss", bufs=2, space="PSUM") as ps_s,
                    tc.tile_pool(name="p2psy", bufs=2, space="PSUM") as ps_y,
                    tc.tile_pool(name="p2psl", bufs=1, space="PSUM") as ps_l,
                ):
                    for h in range(H_LOC if 2 in phases else 0):
                        kv = h // (N_HEADS // N_KV_HEADS)  # local kv head
                        for ib in range(NIB):
                            npairs = 2 * (ib + 1)
                            jmax = 4 * ib + 3
                            y_ps = ps_y.tile([128, 512], F32, tag="y")
                            p_acc = acc_p.tile([128, 512], F32R, tag="pacc")
                            for jp in range(npairs):
                                s_ps = ps_s.tile([128, 2, 512], F32, tag="s")
                                for sj in range(2):
                                    jt = 2 * jp + sj
                                    nc.tensor.matmul(
                                        s_ps[:, sj, :],
                                        kT[:, kv, jt * 128:(jt + 1) * 128],
                                        qT[:, h, ib * 512:(ib + 1) * 512],
                                        start=True, stop=True)
                                pt = pt_p.tile([128, 2, 512], F32R, tag="pt")
                                nc.scalar.activation(
                                    pt, s_ps, mybir.ActivationFunctionType.Exp,
                                    scale=INV_SQRT_HD)
                                if jp >= 2 * ib:  # pair straddles the diagonal
                                    nc.gpsimd.affine_select(
                                        out=pt, in_=pt,
                                        compare_op=mybir.AluOpType.is_ge,
                                        fill=0.0,
                                        base=ib * 512 - 2 * jp * 128,
                                        channel_multiplier=-1,
                                        pattern=[[-128, 2], [1, 512]])
                                if jp == 0:
                                    nc.vector.tensor_add(
                                        p_acc, pt[:, 0, :], pt[:, 1, :])
                                else:
                                    nc.vector.tensor_add(p_acc, p_acc, pt[:, 0, :])
                                    nc.vector.tensor_add(p_acc, p_acc, pt[:, 1, :])
                                for sj in range(2):
                                    jt = 2 * jp + sj
                                    nc.tensor.matmul(
                                        y_ps,
                                        v_sb[:, jt, kv * 128:(kv + 1) * 128],
                                        pt[:, sj, :],
                                        start=(jt == 0), stop=(jt == jmax))

                            l_ps = ps_l.tile([1, 512], F32, tag="l")
                            nc.tensor.matmul(
                                l_ps, ones_128, p_acc, start=True, stop=True)
                            l_sb = r_p.tile([1, 512], F32R, tag="r")
                            nc.vector.tensor_copy(l_sb, l_ps)
                            lb_ps = ps_l.tile([128, 512], F32, tag="lb")
                            nc.tensor.matmul(
                                lb_ps, ones_col, l_sb, start=True, stop=True)
                            linv = ystg_p.tile([128, 512], F32, tag="linv")
                            nc.vector.reciprocal(linv, lb_ps)
                            y_sb = ystg_p.tile([128, 512], F32R, tag="ysb")
                            nc.vector.tensor_mul(y_sb, y_ps, linv)
                            nc.sync.dma_start(
                                ydram[h, :, ib * 512:(ib + 1) * 512], y_sb)

            # ---------------- Phase 3: output projection -----------------
            with (
                tc.tile_pool(name="p3ow", bufs=1) as ow_p,
                tc.tile_pool(name="p3y", bufs=2) as ylhs_p,
                tc.tile_pool(name="p3o", bufs=3) as ostg_p,
                tc.tile_pool(name="p3ps", bufs=4, space="PSUM") as ps_o,
            ):
                ow_sb = ow_p.tile([128, H_LOC, D], F32R)
                nc.gpsimd.dma_start(
                    ow_sb, owt_d.rearrange("(h p) d -> p h d", p=128))
                for it in range(NT if 3 in phases else 0):
                    yl = ylhs_p.tile([128, H_LOC, 128], F32R, tag="yl")
                    nc.gpsimd.dma_start(
                        yl, ydram[:, :, it * 128:(it + 1) * 128]
                        .rearrange("h p t -> p h t"))
                    for db in range(D // 512):
                        o_ps = ps_o.tile([128, 512], F32, tag="o")
                        for h in range(H_LOC):
                            nc.tensor.matmul(
                                o_ps,
                                yl[:, h, :],
                                ow_sb[:, h, db * 512:(db + 1) * 512],
                                start=(h == 0), stop=(h == H_LOC - 1))
                        o_sb = ostg_p.tile([128, 512], F32, tag="osb")
                        nc.vector.tensor_copy(o_sb, o_ps)
                        nc.sync.dma_start(
                            out_d[it * 128:(it + 1) * 128,
                                  db * 512:(db + 1) * 512], o_sb)

    nc.compile()
    return nc


def make_in_maps(x, q_w, k_w, v_w, out_w, q_gain, T):
    cos, sin = _rope_tables(T)
    cost = np.ascontiguousarray(cos.T)  # [64, T]
    sint = np.ascontiguousarray(sin.T)
    cos2 = np.concatenate([cost, cost], axis=0)  # [128, T]
    ssin2 = np.concatenate([sint, -sint], axis=0)
    in_maps = []
    for c in range(8):
        b, g = c // 2, c % 2
        gains = np.concatenate(
            [np.asarray(q_gain[g * H_LOC:(g + 1) * H_LOC], dtype=np.float32),
             np.ones(KV_LOC, dtype=np.float32)])
        in_maps.append({
            "x": np.ascontiguousarray(x[b]),
            "qwt": np.ascontiguousarray(q_w[g * 1024:(g + 1) * 1024, :].T),
            "kwt": np.ascontiguousarray(k_w[g * 256:(g + 1) * 256, :].T),
            "vwt": np.ascontiguousarray(v_w[g * 256:(g + 1) * 256, :].T),
            "owt": np.ascontiguousarray(out_w[:, g * 1024:(g + 1) * 1024].T),
            "cos2": cos2,
            "ssin2": ssin2,
            "gains": gains,
        })
    return in_maps


def kernel(x, q_w, k_w, v_w, out_w, q_gain, _trace=False, _trace_cores=None):
    x = np.asarray(x, dtype=np.float32)
    q_w = np.asarray(q_w, dtype=np.float32)
    k_w = np.asarray(k_w, dtype=np.float32)
    v_w = np.asarray(v_w, dtype=np.float32)
    out_w = np.asarray(out_w, dtype=np.float32)
    q_gain = np.asarray(q_gain, dtype=np.float32)
    T = x.shape[1]

    nc = build_program(T)
    in_maps = make_in_maps(x, q_w, k_w, v_w, out_w, q_gain, T)
    res = bass_utils.run_bass_kernel_spmd(
        nc, in_maps, core_ids=list(range(8)),
        trace=_trace, trace_cores=_trace_cores)
    outs = [r["out"] for r in res.results]
    full = np.stack([outs[2 * b] + outs[2 * b + 1] for b in range(B)])
    if _trace:
        return full.astype(np.float32), res
    return full.astype(np.float32)



# revision 2
# speedup vs baseline: 1.1068x; 1.1068x over previous
"""Causal self-attention (RMS-normed QK, RoPE, GQA) Trainium2 Bass kernel, v2.

Sharding over 8 NeuronCores: 4-way data-parallel over batch x 2-way
tensor-parallel over heads.  Core c handles batch b = c // 2 and head group
g = c % 2 (q heads g*8..g*8+7, kv heads g*2, g*2+1).  Each core produces a
partial output projection; the host sums the two head-group partials per
batch.

v2 changes vs baseline:
  - bf16 data path everywhere (fp32 PSUM accumulation): halves DVE work
    (2x/4x DVE modes for all-bf16-SBUF ops), halves DMA bytes.  Matmul
    throughput is 1 cycle/moving-row either way.
  - x is shipped PRE-TRANSPOSED in bf16 from the host: no on-device x
    transposes (kills 256 PE transposes + 256 DVE copies).
  - Head-dim storage is PERMUTED (consistently in q/k weights and rope
    tables) so the RoPE half-swap (hd -> hd+64) maps to an intra-quadrant
    lane swap, done by a single DVE stream_shuffle instead of SBUF-SBUF
    DMAs.  Scores contract over hd, so the permutation cancels.
  - Causal masking via 4 precomputed [128, 512] mask tiles (host-shipped)
    applied as bf16 DVE multiplies: gpsimd affine_select eliminated.
  - q_gain and the 1/128 mean factor folded into the RMS sqrt via per-head
    scale/bias vectors; sqrt/reciprocal run at [1, 512] before the
    broadcast matmul instead of [128, 512] after it.
  - y stays in SBUF ([hd, t] layout = exactly the out-proj lhsT): the
    ydram spill/reload round trip is gone.
  - PSUM->SBUF staging copies run on the scalar engine (Activation),
    freeing DVE.
"""

import math

import numpy as np
import ml_dtypes

import concourse.bass as bass
import concourse.mybir as mybir
import concourse.tile as tile
from concourse import bacc, bass_utils
from concourse.masks import make_identity

F32 = mybir.dt.float32
F32R = mybir.dt.float32r
BF16 = mybir.dt.bfloat16
AF = mybir.ActivationFunctionType

HEAD_DIM = 128
N_HEADS = 16
N_KV_HEADS = 4
ROPE_BASE = 10000.0
TRAIN_SEQ_LEN = 1024

B, D = 4, 2048
H_LOC = 8  # q heads per core
KV_LOC = 2  # kv heads per core
NH = H_LOC + KV_LOC
EC = D // 128  # contraction chunks
EPS = float(np.finfo(np.float32).eps)
INV_SQRT_HD = 1.0 / math.sqrt(HEAD_DIM)

# Head-dim permutation: lane p = q*32 + l holds original component
#   o = q*16 + l        (l < 16)
#   o = 64 + q*16 + l-16 (l >= 16)
# so the rope partner (o <-> o+64) sits 16 lanes away in the same
# 32-lane quadrant; SWAP_MASK swaps the two 16-lane halves.
PERM = np.array(
    [q * 16 + l if l < 16 else 64 + q * 16 + (l - 16)
     for q in range(4) for l in range(32)],
    dtype=np.int64,
)
SWAP_MASK = [(l + 16) % 32 for l in range(32)]


def _rope_tables(T):
    rd = HEAD_DIM
    base = ROPE_BASE
    if T > TRAIN_SEQ_LEN:
        scale = T / TRAIN_SEQ_LEN
        base = base * scale ** (rd / (rd - 2))
    inv_freq = 1.0 / base ** (np.arange(0, rd, 2, dtype=np.float32) / rd)
    freqs = np.outer(np.arange(T, dtype=np.float32), inv_freq)
    return np.cos(freqs).astype(np.float32), np.sin(freqs).astype(np.float32)


def build_program(T=2048, phases=(1, 2, 3)):
    """Build the per-core Bass program. T must be a multiple of 512."""
    assert T % 512 == 0
    NT = T // 128  # 128-row t tiles
    NTB = T // 512  # 512-col t blocks
    NIB = T // 512  # attention i blocks

    nc = bacc.Bacc("TRN2", target_bir_lowering=False, debug=False, num_devices=8)

    # All bf16 inputs ship as ONE flat blob (the per-call dispatch cost
    # scales with the number of PJRT buffers, ~90us/buffer/exec via axon).
    sizes = {
        "xt": D * T,
        "qwt": D * H_LOC * 128,
        "kwt": D * KV_LOC * 128,
        "vwt": D * KV_LOC * 128,
        "owt": H_LOC * 128 * D,
        "cosp": 128 * T,
        "sinp": 128 * T,
        "masks": 128 * 4 * 512,
    }
    offs, o = {}, 0
    for nm, sz in sizes.items():
        offs[nm] = o
        o += sz
    blob_d = nc.dram_tensor("blob", [o], BF16, kind="ExternalInput").ap()

    def bseg(nm, shape):
        ap = blob_d[offs[nm]:offs[nm] + sizes[nm]]
        pat = " ".join(f"d{i}" for i in range(len(shape)))
        kw = {f"d{i}": s for i, s in enumerate(shape)}
        return ap.rearrange(f"({pat}) -> {pat}", **kw)

    qwt_d = bseg("qwt", [D, H_LOC * 128])
    kwt_d = bseg("kwt", [D, KV_LOC * 128])
    vwt_d = bseg("vwt", [D, KV_LOC * 128])
    # per-(q|k)-head rms factors: row 0 sc = 1/(128*g^2), row 1 bi = eps/g^2
    scbi_d = nc.dram_tensor("scbi", [2, NH], F32, kind="ExternalInput").ap()
    out_d = nc.dram_tensor("out", [T, D], F32, kind="ExternalOutput").ap()

    with tile.TileContext(nc) as tc:
        with (
            tc.tile_pool(name="const", bufs=1) as const_p,
            tc.tile_pool(name="persist", bufs=1) as pers_p,
        ):
            ident = const_p.tile([128, 128], BF16)
            make_identity(nc, ident)
            ones_col = const_p.tile([1, 128], F32R)  # lhsT for K=1 bcast matmul
            nc.vector.memset(ones_col.bitcast(F32), 1.0)
            ones_128 = const_p.tile([128, 1], F32R)  # lhsT for partition-sum
            nc.vector.memset(ones_128.bitcast(F32), 1.0)
            ones_128_bf = const_p.tile([128, 1], BF16)  # same, for bf16 rhs
            nc.vector.memset(ones_128_bf, 1.0)
            masks_sb = const_p.tile([128, 4, 512], BF16)
            nc.sync.dma_start(masks_sb, bseg("masks", [128, 4, 512]))
            sc_sb = const_p.tile([1, NH], F32)
            nc.sync.dma_start(sc_sb, scbi_d[0:1, :])
            bi_sb = const_p.tile([1, NH], F32)
            nc.sync.dma_start(bi_sb, scbi_d[1:2, :])

            qT = pers_p.tile([128, H_LOC, T], BF16)
            kT = pers_p.tile([128, KV_LOC, T], BF16)
            v_sb = pers_p.tile([128, NT, KV_LOC * 128], BF16)
            y_sb = pers_p.tile([128, H_LOC, T], BF16)

            # ---------------- Phase 1: projections -------------------
            with (
                tc.tile_pool(name="p1xt", bufs=1) as xt_p,
                tc.tile_pool(name="p1rope", bufs=1) as rope_p,
                tc.tile_pool(name="p1w", bufs=2) as w_p,
                tc.tile_pool(name="p1work", bufs=3) as wk_p,
                tc.tile_pool(name="p1workf", bufs=2) as wkf_p,
                tc.tile_pool(name="p1stat", bufs=2) as st_p,
                tc.tile_pool(name="p1ps", bufs=2, space="PSUM") as ps_p,
                tc.tile_pool(name="p1pss", bufs=2, space="PSUM") as ps_s,
                tc.tile_pool(name="p1psb", bufs=2, space="PSUM") as ps_b,
                tc.tile_pool(name="p1pst", bufs=2, space="PSUM") as ps_t,
            ):
                cos_sb = rope_p.tile([128, T], BF16)
                nc.sync.dma_start(cos_sb, bseg("cosp", [128, T]))
                sin_sb = rope_p.tile([128, T], BF16)
                nc.sync.dma_start(sin_sb, bseg("sinp", [128, T]))
                xt = xt_p.tile([128, EC, T], BF16)
                xt_r = bseg("xt", [D, T]).rearrange("(e p) t -> p e t", p=128)
                nc.sync.dma_start(xt[:, :EC // 2], xt_r[:, :EC // 2])
                nc.sync.dma_start(xt[:, EC // 2:], xt_r[:, EC // 2:])

                # column schedule: (wtensor, local col, gain idx, kind)
                # v first so v_sb is complete early; q last heads last.
                cols = []
                for kv in range(KV_LOC):
                    cols.append((vwt_d, kv, None, "v"))
                for kv in range(KV_LOC):
                    cols.append((kwt_d, kv, H_LOC + kv, "k"))
                for h in range(H_LOC):
                    cols.append((qwt_d, h, h, "q"))

                # load weights in column pairs (512B contiguous runs)
                for ci in range(0, len(cols), 2):
                    pair = cols[ci:ci + 2]
                    w_dram = pair[0][0]
                    c0 = pair[0][1] * 128
                    assert len(pair) == 2 and pair[1][0] is w_dram
                    wt = w_p.tile([128, EC, 256], BF16, tag="w")
                    nc.sync.dma_start(
                        wt, w_dram[:, c0:c0 + 256]
                        .rearrange("(e p) c -> p e c", p=128))

                    for wi, (_, lc, gi, kind) in enumerate(pair):
                        for tb in range(NTB):
                            tsl = slice(tb * 512, (tb + 1) * 512)
                            h_ps = ps_p.tile([128, 512], F32, tag="proj")
                            for ecc in range(EC):
                                nc.tensor.matmul(
                                    h_ps, wt[:, ecc, wi * 128:(wi + 1) * 128],
                                    xt[:, ecc, tsl],
                                    start=(ecc == 0), stop=(ecc == EC - 1))
                            if kind == "v":
                                vt = wk_p.tile([128, 512], BF16, tag="vt")
                                nc.scalar.activation(vt, h_ps, AF.Copy)
                                for tt in range(4):
                                    pst = ps_t.tile([128, 128], BF16, tag="tp")
                                    nc.tensor.transpose(
                                        pst, vt[:, tt * 128:(tt + 1) * 128],
                                        ident)
                                    nc.vector.tensor_copy(
                                        v_sb[:, tb * 4 + tt,
                                             lc * 128:(lc + 1) * 128], pst)
                                continue
                            # rms norm: ssq over partitions, rinv = g/sqrt(
                            # mean+eps) computed at [1,512], then broadcast
                            sq = wkf_p.tile([128, 512], F32R, tag="sq")
                            nc.scalar.activation(sq, h_ps, AF.Square)
                            ssq_ps = ps_s.tile([1, 512], F32, tag="ssq")
                            nc.tensor.matmul(
                                ssq_ps, ones_128, sq, start=True, stop=True)
                            rms = st_p.tile([1, 512], F32R, tag="rms")
                            nc.scalar.activation(
                                rms, ssq_ps, AF.Sqrt,
                                bias=bi_sb[0:1, gi:gi + 1],
                                scale=sc_sb[0:1, gi:gi + 1])
                            rinv = st_p.tile([1, 512], F32R, tag="rinv")
                            with nc.allow_low_precision(
                                    reason="f32r tag for full-rate matmul"):
                                nc.vector.reciprocal(rinv, rms)
                            rinv_ps = ps_b.tile([128, 512], F32, tag="rb")
                            nc.tensor.matmul(
                                rinv_ps, ones_col, rinv, start=True, stop=True)
                            rinvb = wkf_p.tile([128, 512], F32R, tag="rbs")
                            nc.scalar.activation(rinvb, rinv_ps, AF.Copy)
                            qn = wk_p.tile([128, 512], BF16, tag="qn")
                            nc.vector.tensor_mul(qn, h_ps, rinvb)
                            # rope: dst = qn*cos + shuffle(qn)*sin
                            qsw = wk_p.tile([128, 512], BF16, tag="qsw")
                            nc.vector.stream_shuffle(qsw, qn, SWAP_MASK)
                            rc = wk_p.tile([128, 512], BF16, tag="rc")
                            nc.vector.tensor_mul(rc, qn, cos_sb[:, tsl])
                            qs = wk_p.tile([128, 512], BF16, tag="qs")
                            nc.vector.tensor_mul(qs, qsw, sin_sb[:, tsl])
                            dst = (qT[:, lc, tsl] if kind == "q"
                                   else kT[:, lc, tsl])
                            nc.vector.tensor_add(dst, rc, qs)

            # ---------------- Phase 2: attention ---------------------
            with tc.tile_pool(name="p3ow", bufs=1) as ow_p:
                ow_sb = ow_p.tile([128, H_LOC, D], BF16)
                nc.sync.dma_start(
                    ow_sb,
                    bseg("owt", [H_LOC * 128, D])
                    .rearrange("(h p) d -> p h d", p=128))

                with (
                    tc.tile_pool(name="p2pt", bufs=5) as pt_p,
                    tc.tile_pool(name="p2acc", bufs=2) as acc_p,
                    tc.tile_pool(name="p2r", bufs=2) as r_p,
                    tc.tile_pool(name="p2pss", bufs=2, space="PSUM") as ps_s2,
                    tc.tile_pool(name="p2psy", bufs=2, space="PSUM") as ps_y,
                    tc.tile_pool(name="p2psl", bufs=1, space="PSUM") as ps_l,
                    tc.tile_pool(name="p2pslb", bufs=1, space="PSUM") as ps_lb,
                ):
                    for h in range(H_LOC if 2 in phases else 0):
                        kv = h // (N_HEADS // N_KV_HEADS)  # local kv head
                        for ib in range(NIB):
                            npairs = 2 * (ib + 1)
                            jmax = 4 * ib + 3
                            y_ps = ps_y.tile([128, 512], F32, tag="y")
                            p_acc = acc_p.tile([128, 512], F32R, tag="pacc")
                            l_ps = ps_l.tile([1, 512], F32, tag="l")
                            l_started = False
                            for jp in range(npairs):
                                s_ps = ps_s2.tile([128, 2, 512], F32, tag="s")
                                for sj in range(2):
                                    jt = 2 * jp + sj
                                    nc.tensor.matmul(
                                        s_ps[:, sj, :],
                                        kT[:, kv, jt * 128:(jt + 1) * 128],
                                        qT[:, h, ib * 512:(ib + 1) * 512],
                                        start=True, stop=True)
                                pt = pt_p.tile([128, 2, 512], BF16, tag="pt")
                                nc.scalar.activation(
                                    pt, s_ps, AF.Exp, scale=INV_SQRT_HD)
                                if jp >= 2 * ib:  # pair straddles the diagonal
                                    dlt = jp - 2 * ib
                                    for sj in range(2):
                                        nc.vector.tensor_mul(
                                            pt[:, sj, :], pt[:, sj, :],
                                            masks_sb[:, 2 * dlt + sj, :])
                                # denominator: off-diagonal odd pairs summed
                                # on PE (PSUM-accumulated ones-matmuls into
                                # l_ps), the rest on DVE into p_acc (bf16
                                # pair pre-sum, fp32 accumulate); l_ps gets
                                # p_acc's total at the end.
                                if jp % 2 == 1 and jp < 2 * ib:
                                    for sj in range(2):
                                        nc.tensor.matmul(
                                            l_ps, ones_128_bf, pt[:, sj, :],
                                            start=(not l_started), stop=False)
                                        l_started = True
                                elif jp == 0:
                                    nc.vector.tensor_add(
                                        p_acc, pt[:, 0, :], pt[:, 1, :])
                                else:
                                    t1 = pt_p.tile([128, 512], BF16, tag="t1")
                                    nc.vector.tensor_add(
                                        t1, pt[:, 0, :], pt[:, 1, :])
                                    nc.vector.tensor_add(p_acc, p_acc, t1)
                                for sj in range(2):
                                    jt = 2 * jp + sj
                                    nc.tensor.matmul(
                                        y_ps,
                                        v_sb[:, jt, kv * 128:(kv + 1) * 128],
                                        pt[:, sj, :],
                                        start=(jt == 0), stop=(jt == jmax))

                            nc.tensor.matmul(
                                l_ps, ones_128, p_acc,
                                start=(not l_started), stop=True)
                            lr = r_p.tile([1, 512], F32R, tag="lr")
                            with nc.allow_low_precision(
                                    reason="f32r tag for full-rate matmul"):
                                nc.vector.reciprocal(lr, l_ps)
                            linv_ps = ps_lb.tile([128, 512], F32, tag="lb")
                            nc.tensor.matmul(
                                linv_ps, ones_col, lr, start=True, stop=True)
                            linvb = r_p.tile([128, 512], F32R, tag="lbs")
                            nc.scalar.activation(linvb, linv_ps, AF.Copy)
                            nc.vector.tensor_mul(
                                y_sb[:, h, ib * 512:(ib + 1) * 512],
                                y_ps, linvb)

                # ---------------- Phase 3: output projection ---------
                with (
                    tc.tile_pool(name="p3o", bufs=3) as ostg_p,
                    tc.tile_pool(name="p3ps", bufs=4, space="PSUM") as ps_o,
                ):
                    for it in range(NT if 3 in phases else 0):
                        for db in range(D // 512):
                            o_ps = ps_o.tile([128, 512], F32, tag="o")
                            for h in range(H_LOC):
                                nc.tensor.matmul(
                                    o_ps,
                                    y_sb[:, h, it * 128:(it + 1) * 128],
                                    ow_sb[:, h, db * 512:(db + 1) * 512],
                                    start=(h == 0), stop=(h == H_LOC - 1))
                            o_sb = ostg_p.tile([128, 512], F32, tag="osb")
                            nc.scalar.activation(o_sb, o_ps, AF.Copy)
                            nc.sync.dma_start(
                                out_d[it * 128:(it + 1) * 128,
                                      db * 512:(db + 1) * 512], o_sb)

    nc.compile()
    return nc


def make_in_maps(x, q_w, k_w, v_w, out_w, q_gain, T):
    bf16 = ml_dtypes.bfloat16
    cos, sin = _rope_tables(T)  # [T, 64]
    # permuted rope tables [128, T]
    cosp = np.empty((128, T), dtype=np.float32)
    sinp = np.empty((128, T), dtype=np.float32)
    for p in range(128):
        o = PERM[p]
        if o < 64:
            cosp[p] = cos[:, o]
            sinp[p] = sin[:, o]
        else:
            cosp[p] = cos[:, o - 64]
            sinp[p] = -sin[:, o - 64]
    cosp = cosp.astype(bf16)
    sinp = sinp.astype(bf16)

    # diagonal-block causal masks: mask[p, m, c] = (c >= m*128 + p)
    cc = np.arange(512)[None, None, :]
    mm = np.arange(4)[None, :, None]
    pp = np.arange(128)[:, None, None]
    masks = (cc >= mm * 128 + pp).astype(bf16)

    # permute head-dim columns within each head of a [D, nh*128] W^T matrix
    def permute_cols(wt, nh):
        idx = np.concatenate([h * 128 + PERM for h in range(nh)])
        return wt[:, idx]

    in_maps = []
    for c in range(8):
        b, g = c // 2, c % 2
        gains = np.concatenate(
            [np.asarray(q_gain[g * H_LOC:(g + 1) * H_LOC], dtype=np.float32),
             np.ones(KV_LOC, dtype=np.float32)])
        sc = 1.0 / (128.0 * gains * gains)
        bi = EPS / (gains * gains)
        qwt = permute_cols(
            np.ascontiguousarray(q_w[g * 1024:(g + 1) * 1024, :].T), H_LOC)
        kwt = permute_cols(
            np.ascontiguousarray(k_w[g * 256:(g + 1) * 256, :].T), KV_LOC)
        # single blob in the exact order build_program's `sizes` declares
        blob = np.concatenate([
            np.ascontiguousarray(x[b].T).astype(bf16).ravel(),
            qwt.astype(bf16).ravel(),
            kwt.astype(bf16).ravel(),
            np.ascontiguousarray(
                v_w[g * 256:(g + 1) * 256, :].T).astype(bf16).ravel(),
            np.ascontiguousarray(
                out_w[:, g * 1024:(g + 1) * 1024].T).astype(bf16).ravel(),
            cosp.ravel(),
            sinp.ravel(),
            masks.ravel(),
        ])
        in_maps.append({
            "blob": blob,
            "scbi": np.stack([sc, bi]).astype(np.float32),
        })
    return in_maps


def kernel(x, q_w, k_w, v_w, out_w, q_gain, _trace=False, _trace_cores=None):
    x = np.asarray(x, dtype=np.float32)
    q_w = np.asarray(q_w, dtype=np.float32)
    k_w = np.asarray(k_w, dtype=np.float32)
    v_w = np.asarray(v_w, dtype=np.float32)
    out_w = np.asarray(out_w, dtype=np.float32)
    q_gain = np.asarray(q_gain, dtype=np.float32)
    T = x.shape[1]

    nc = build_program(T)
    in_maps = make_in_maps(x, q_w, k_w, v_w, out_w, q_gain, T)
    res = bass_utils.run_bass_kernel_spmd(
        nc, in_maps, core_ids=list(range(8)),
        trace=_trace, trace_cores=_trace_cores)
    outs = [r["out"] for r in res.results]
    full = np.stack([outs[2 * b] + outs[2 * b + 1] for b in range(B)])
    if _trace:
        return full.astype(np.float32), res
    return full.astype(np.float32)


# revision 7
# speedup vs baseline: 1.1913x; 1.0764x over previous
"""Causal self-attention (RMS-normed QK, RoPE, GQA) Trainium2 Bass kernel, v2.

Sharding over 8 NeuronCores: 4-way data-parallel over batch x 2-way
tensor-parallel over heads.  Core c handles batch b = c // 2 and head group
g = c % 2 (q heads g*8..g*8+7, kv heads g*2, g*2+1).  Each core produces a
partial output projection; the host sums the two head-group partials per
batch.

v2 changes vs baseline:
  - bf16 data path everywhere (fp32 PSUM accumulation): halves DVE work
    (2x/4x DVE modes for all-bf16-SBUF ops), halves DMA bytes.  Matmul
    throughput is 1 cycle/moving-row either way.
  - x is shipped PRE-TRANSPOSED in bf16 from the host: no on-device x
    transposes (kills 256 PE transposes + 256 DVE copies).
  - Head-dim storage is PERMUTED (consistently in q/k weights and rope
    tables) so the RoPE half-swap (hd -> hd+64) maps to an intra-quadrant
    lane swap, done by a single DVE stream_shuffle instead of SBUF-SBUF
    DMAs.  Scores contract over hd, so the permutation cancels.
  - Causal masking via 4 precomputed [128, 512] mask tiles (host-shipped)
    applied as bf16 DVE multiplies: gpsimd affine_select eliminated.
  - q_gain and the 1/128 mean factor folded into the RMS sqrt via per-head
    scale/bias vectors; sqrt/reciprocal run at [1, 512] before the
    broadcast matmul instead of [128, 512] after it.
  - y stays in SBUF ([hd, t] layout = exactly the out-proj lhsT): the
    ydram spill/reload round trip is gone.
  - PSUM->SBUF staging copies run on the scalar engine (Activation),
    freeing DVE.
"""

import math

import numpy as np
import ml_dtypes

import concourse.bass as bass
import concourse.mybir as mybir
import concourse.tile as tile
from concourse import bacc, bass_utils
from concourse.masks import make_identity

F32 = mybir.dt.float32
F32R = mybir.dt.float32r
BF16 = mybir.dt.bfloat16
AF = mybir.ActivationFunctionType

HEAD_DIM = 128
N_HEADS = 16
N_KV_HEADS = 4
ROPE_BASE = 10000.0
TRAIN_SEQ_LEN = 1024

B, D = 4, 2048
H_LOC = 8  # q heads per core
KV_LOC = 2  # kv heads per core
NH = H_LOC + KV_LOC
EC = D // 128  # contraction chunks
EPS = float(np.finfo(np.float32).eps)
INV_SQRT_HD = 1.0 / math.sqrt(HEAD_DIM)

# Head-dim permutation: lane p = q*32 + l holds original component
#   o = q*16 + l        (l < 16)
#   o = 64 + q*16 + l-16 (l >= 16)
# so the rope partner (o <-> o+64) sits 16 lanes away in the same
# 32-lane quadrant; SWAP_MASK swaps the two 16-lane halves.
PERM = np.array(
    [q * 16 + l if l < 16 else 64 + q * 16 + (l - 16)
     for q in range(4) for l in range(32)],
    dtype=np.int64,
)
SWAP_MASK = [(l + 16) % 32 for l in range(32)]


def _rope_tables(T):
    rd = HEAD_DIM
    base = ROPE_BASE
    if T > TRAIN_SEQ_LEN:
        scale = T / TRAIN_SEQ_LEN
        base = base * scale ** (rd / (rd - 2))
    inv_freq = 1.0 / base ** (np.arange(0, rd, 2, dtype=np.float32) / rd)
    freqs = np.outer(np.arange(T, dtype=np.float32), inv_freq)
    return np.cos(freqs).astype(np.float32), np.sin(freqs).astype(np.float32)


def build_program(T=2048, phases=(1, 2, 3)):
    """Build the per-core Bass program. T must be a multiple of 512."""
    assert T % 512 == 0
    NT = T // 128  # 128-row t tiles
    NTB = T // 512  # 512-col t blocks
    NIB = T // 512  # attention i blocks

    nc = bacc.Bacc("TRN2", target_bir_lowering=False, debug=False, num_devices=8)

    # All bf16 inputs ship as ONE flat blob (the per-call dispatch cost
    # scales with the number of PJRT buffers, ~90us/buffer/exec via axon).
    sizes = {
        "xt": D * T,
        "qwt": D * H_LOC * 128,
        "kwt": D * KV_LOC * 128,
        "vwt": D * KV_LOC * 128,
        "owt": H_LOC * 128 * D,
        "cosp": 128 * T,
        "sinp": 128 * T,
        "masks": 128 * 4 * 512,
        # rms factors in bf16: cols 0..NH-1 sc = 1/(128*g^2), cols
        # NH..2NH-1 bi = eps/g^2 (bf16-exact for g=1: 2^-7 and 2^-23)
        "scbi": 2 * NH,
    }
    offs, o = {}, 0
    for nm, sz in sizes.items():
        offs[nm] = o
        o += sz
    blob_d = nc.dram_tensor("blob", [o], BF16, kind="ExternalInput").ap()

    def bseg(nm, shape):
        ap = blob_d[offs[nm]:offs[nm] + sizes[nm]]
        pat = " ".join(f"d{i}" for i in range(len(shape)))
        kw = {f"d{i}": s for i, s in enumerate(shape)}
        return ap.rearrange(f"({pat}) -> {pat}", **kw)

    qwt_d = bseg("qwt", [D, H_LOC * 128])
    kwt_d = bseg("kwt", [D, KV_LOC * 128])
    vwt_d = bseg("vwt", [D, KV_LOC * 128])
    out_d = nc.dram_tensor("out", [T, D], F32, kind="ExternalOutput").ap()

    with tile.TileContext(nc) as tc:
        with (
            tc.tile_pool(name="const", bufs=1) as const_p,
            tc.tile_pool(name="persist", bufs=1) as pers_p,
        ):
            ident = const_p.tile([128, 128], BF16)
            make_identity(nc, ident)
            ones_col = const_p.tile([1, 128], F32R)  # lhsT for K=1 bcast matmul
            nc.vector.memset(ones_col.bitcast(F32), 1.0)
            ones_128 = const_p.tile([128, 1], F32R)  # lhsT for partition-sum
            nc.vector.memset(ones_128.bitcast(F32), 1.0)
            ones_128_bf = const_p.tile([128, 1], BF16)  # same, for bf16 rhs
            nc.vector.memset(ones_128_bf, 1.0)
            masks_sb = const_p.tile([128, 4, 512], BF16)
            nc.sync.dma_start(masks_sb, bseg("masks", [128, 4, 512]))
            scbi_bf = const_p.tile([1, 2 * NH], BF16)
            nc.sync.dma_start(scbi_bf, bseg("scbi", [1, 2 * NH]))
            scbi_f = const_p.tile([1, 2 * NH], F32)
            nc.scalar.activation(scbi_f, scbi_bf, AF.Copy)

            qT = pers_p.tile([128, H_LOC, T], BF16)
            kT = pers_p.tile([128, KV_LOC, T], BF16)
            v_sb = pers_p.tile([128, NT, KV_LOC * 128], BF16)
            y_sb = pers_p.tile([128, H_LOC, T], BF16)

            # ---------------- Phase 1: projections -------------------
            with (
                tc.tile_pool(name="p1xt", bufs=1) as xt_p,
                tc.tile_pool(name="p1rope", bufs=1) as rope_p,
                tc.tile_pool(name="p1w", bufs=2) as w_p,
                tc.tile_pool(name="p1work", bufs=3) as wk_p,
                tc.tile_pool(name="p1workf", bufs=2) as wkf_p,
                tc.tile_pool(name="p1stat", bufs=2) as st_p,
                tc.tile_pool(name="p1ps", bufs=2, space="PSUM") as ps_p,
                tc.tile_pool(name="p1pss", bufs=2, space="PSUM") as ps_s,
                tc.tile_pool(name="p1psb", bufs=2, space="PSUM") as ps_b,
                tc.tile_pool(name="p1pst", bufs=2, space="PSUM") as ps_t,
            ):
                cos_sb = rope_p.tile([128, T], BF16)
                nc.sync.dma_start(cos_sb, bseg("cosp", [128, T]))
                sin_sb = rope_p.tile([128, T], BF16)
                nc.sync.dma_start(sin_sb, bseg("sinp", [128, T]))
                xt = xt_p.tile([128, EC, T], BF16)
                xt_r = bseg("xt", [D, T]).rearrange("(e p) t -> p e t", p=128)
                nc.sync.dma_start(xt[:, :EC // 2], xt_r[:, :EC // 2])
                nc.sync.dma_start(xt[:, EC // 2:], xt_r[:, EC // 2:])

                # column schedule: (wtensor, local col, gain idx, kind)
                # v first so v_sb is complete early; q last heads last.
                cols = []
                for kv in range(KV_LOC):
                    cols.append((vwt_d, kv, None, "v"))
                for kv in range(KV_LOC):
                    cols.append((kwt_d, kv, H_LOC + kv, "k"))
                for h in range(H_LOC):
                    cols.append((qwt_d, h, h, "q"))

                # load weights in column pairs (512B contiguous runs)
                for ci in range(0, len(cols), 2):
                    pair = cols[ci:ci + 2]
                    w_dram = pair[0][0]
                    c0 = pair[0][1] * 128
                    assert len(pair) == 2 and pair[1][0] is w_dram
                    wt = w_p.tile([128, EC, 256], BF16, tag="w")
                    nc.sync.dma_start(
                        wt, w_dram[:, c0:c0 + 256]
                        .rearrange("(e p) c -> p e c", p=128))

                    for wi, (_, lc, gi, kind) in enumerate(pair):
                        for tb in range(NTB):
                            tsl = slice(tb * 512, (tb + 1) * 512)
                            h_ps = ps_p.tile([128, 512], F32, tag="proj")
                            for ecc in range(EC):
                                nc.tensor.matmul(
                                    h_ps, wt[:, ecc, wi * 128:(wi + 1) * 128],
                                    xt[:, ecc, tsl],
                                    start=(ecc == 0), stop=(ecc == EC - 1))
                            if kind == "v":
                                vt = wk_p.tile([128, 512], BF16, tag="vt")
                                nc.scalar.activation(vt, h_ps, AF.Copy)
                                for tt in range(4):
                                    pst = ps_t.tile([128, 128], BF16, tag="tp")
                                    nc.tensor.transpose(
                                        pst, vt[:, tt * 128:(tt + 1) * 128],
                                        ident)
                                    nc.vector.tensor_copy(
                                        v_sb[:, tb * 4 + tt,
                                             lc * 128:(lc + 1) * 128], pst)
                                continue
                            # rms norm: ssq over partitions, rinv = g/sqrt(
                            # mean+eps) computed at [1,512], then broadcast
                            sq = wkf_p.tile([128, 512], F32R, tag="sq")
                            nc.scalar.activation(sq, h_ps, AF.Square)
                            ssq_ps = ps_s.tile([1, 512], F32, tag="ssq")
                            nc.tensor.matmul(
                                ssq_ps, ones_128, sq, start=True, stop=True)
                            rms = st_p.tile([1, 512], F32R, tag="rms")
                            nc.scalar.activation(
                                rms, ssq_ps, AF.Sqrt,
                                bias=scbi_f[0:1, NH + gi:NH + gi + 1],
                                scale=scbi_f[0:1, gi:gi + 1])
                            rinv = st_p.tile([1, 512], F32R, tag="rinv")
                            with nc.allow_low_precision(
                                    reason="f32r tag for full-rate matmul"):
                                nc.vector.reciprocal(rinv, rms)
                            rinv_ps = ps_b.tile([128, 512], F32, tag="rb")
                            nc.tensor.matmul(
                                rinv_ps, ones_col, rinv, start=True, stop=True)
                            rinvb = wkf_p.tile([128, 512], F32R, tag="rbs")
                            nc.scalar.activation(rinvb, rinv_ps, AF.Copy)
                            qn = wk_p.tile([128, 512], BF16, tag="qn")
                            nc.vector.tensor_mul(qn, h_ps, rinvb)
                            # rope: dst = qn*cos + shuffle(qn)*sin
                            qsw = wk_p.tile([128, 512], BF16, tag="qsw")
                            nc.vector.stream_shuffle(qsw, qn, SWAP_MASK)
                            rc = wk_p.tile([128, 512], BF16, tag="rc")
                            nc.vector.tensor_mul(rc, qn, cos_sb[:, tsl])
                            qs = wk_p.tile([128, 512], BF16, tag="qs")
                            nc.vector.tensor_mul(qs, qsw, sin_sb[:, tsl])
                            dst = (qT[:, lc, tsl] if kind == "q"
                                   else kT[:, lc, tsl])
                            nc.vector.tensor_add(dst, rc, qs)

            # ---------------- Phase 2: attention ---------------------
            with tc.tile_pool(name="p3ow", bufs=1) as ow_p:
                ow_sb = ow_p.tile([128, H_LOC, D], BF16)
                nc.sync.dma_start(
                    ow_sb,
                    bseg("owt", [H_LOC * 128, D])
                    .rearrange("(h p) d -> p h d", p=128))

                with (
                    tc.tile_pool(name="p2pt", bufs=5) as pt_p,
                    tc.tile_pool(name="p2acc", bufs=2) as acc_p,
                    tc.tile_pool(name="p2r", bufs=2) as r_p,
                    tc.tile_pool(name="p2pss", bufs=2, space="PSUM") as ps_s2,
                    tc.tile_pool(name="p2psy", bufs=2, space="PSUM") as ps_y,
                    tc.tile_pool(name="p2psl", bufs=1, space="PSUM") as ps_l,
                    tc.tile_pool(name="p2pslb", bufs=1, space="PSUM") as ps_lb,
                ):
                    for h in range(H_LOC if 2 in phases else 0):
                        kv = h // (N_HEADS // N_KV_HEADS)  # local kv head
                        for ib in range(NIB):
                            npairs = 2 * (ib + 1)
                            jmax = 4 * ib + 3
                            y_ps = ps_y.tile([128, 512], F32, tag="y")
                            p_acc = acc_p.tile([128, 512], F32R, tag="pacc")
                            l_ps = ps_l.tile([1, 512], F32, tag="l")
                            l_started = False
                            for jp in range(npairs):
                                s_ps = ps_s2.tile([128, 2, 512], F32, tag="s")
                                for sj in range(2):
                                    jt = 2 * jp + sj
                                    nc.tensor.matmul(
                                        s_ps[:, sj, :],
                                        kT[:, kv, jt * 128:(jt + 1) * 128],
                                        qT[:, h, ib * 512:(ib + 1) * 512],
                                        start=True, stop=True)
                                pt = pt_p.tile([128, 2, 512], BF16, tag="pt")
                                nc.scalar.activation(
                                    pt, s_ps, AF.Exp, scale=INV_SQRT_HD)
                                if jp >= 2 * ib:  # pair straddles the diagonal
                                    dlt = jp - 2 * ib
                                    for sj in range(2):
                                        nc.vector.tensor_mul(
                                            pt[:, sj, :], pt[:, sj, :],
                                            masks_sb[:, 2 * dlt + sj, :])
                                # denominator: off-diagonal odd pairs summed
                                # on PE (PSUM-accumulated ones-matmuls into
                                # l_ps), the rest on DVE into p_acc (bf16
                                # pair pre-sum, fp32 accumulate); l_ps gets
                                # p_acc's total at the end.
                                if jp % 2 == 1 and jp < 2 * ib:
                                    for sj in range(2):
                                        nc.tensor.matmul(
                                            l_ps, ones_128_bf, pt[:, sj, :],
                                            start=(not l_started), stop=False)
                                        l_started = True
                                elif jp == 0:
                                    nc.vector.tensor_add(
                                        p_acc, pt[:, 0, :], pt[:, 1, :])
                                else:
                                    t1 = pt_p.tile([128, 512], BF16, tag="t1")
                                    nc.vector.tensor_add(
                                        t1, pt[:, 0, :], pt[:, 1, :])
                                    nc.vector.tensor_add(p_acc, p_acc, t1)
                                for sj in range(2):
                                    jt = 2 * jp + sj
                                    nc.tensor.matmul(
                                        y_ps,
                                        v_sb[:, jt, kv * 128:(kv + 1) * 128],
                                        pt[:, sj, :],
                                        start=(jt == 0), stop=(jt == jmax))

                            nc.tensor.matmul(
                                l_ps, ones_128, p_acc,
                                start=(not l_started), stop=True)
                            lr = r_p.tile([1, 512], F32R, tag="lr")
                            with nc.allow_low_precision(
                                    reason="f32r tag for full-rate matmul"):
                                nc.vector.reciprocal(lr, l_ps)
                            linv_ps = ps_lb.tile([128, 512], F32, tag="lb")
                            nc.tensor.matmul(
                                linv_ps, ones_col, lr, start=True, stop=True)
                            linvb = r_p.tile([128, 512], F32R, tag="lbs")
                            nc.scalar.activation(linvb, linv_ps, AF.Copy)
                            nc.vector.tensor_mul(
                                y_sb[:, h, ib * 512:(ib + 1) * 512],
                                y_ps, linvb)

                # ---------------- Phase 3: output projection ---------
                with (
                    tc.tile_pool(name="p3o", bufs=3) as ostg_p,
                    tc.tile_pool(name="p3ps", bufs=4, space="PSUM") as ps_o,
                ):
                    for it in range(NT if 3 in phases else 0):
                        for db in range(D // 512):
                            o_ps = ps_o.tile([128, 512], F32, tag="o")
                            for h in range(H_LOC):
                                nc.tensor.matmul(
                                    o_ps,
                                    y_sb[:, h, it * 128:(it + 1) * 128],
                                    ow_sb[:, h, db * 512:(db + 1) * 512],
                                    start=(h == 0), stop=(h == H_LOC - 1))
                            o_sb = ostg_p.tile([128, 512], F32, tag="osb")
                            nc.scalar.activation(o_sb, o_ps, AF.Copy)
                            nc.sync.dma_start(
                                out_d[it * 128:(it + 1) * 128,
                                      db * 512:(db + 1) * 512], o_sb)

    nc.compile()
    return nc


def make_in_maps(x, q_w, k_w, v_w, out_w, q_gain, T):
    bf16 = ml_dtypes.bfloat16
    cos, sin = _rope_tables(T)  # [T, 64]
    # permuted rope tables [128, T]
    cosp = np.empty((128, T), dtype=np.float32)
    sinp = np.empty((128, T), dtype=np.float32)
    for p in range(128):
        o = PERM[p]
        if o < 64:
            cosp[p] = cos[:, o]
            sinp[p] = sin[:, o]
        else:
            cosp[p] = cos[:, o - 64]
            sinp[p] = -sin[:, o - 64]
    cosp = cosp.astype(bf16)
    sinp = sinp.astype(bf16)

    # diagonal-block causal masks: mask[p, m, c] = (c >= m*128 + p)
    cc = np.arange(512)[None, None, :]
    mm = np.arange(4)[None, :, None]
    pp = np.arange(128)[:, None, None]
    masks = (cc >= mm * 128 + pp).astype(bf16)

    # permute head-dim columns within each head of a [D, nh*128] W^T matrix
    def permute_cols(wt, nh):
        idx = np.concatenate([h * 128 + PERM for h in range(nh)])
        return wt[:, idx]

    in_maps = []
    for c in range(8):
        b, g = c // 2, c % 2
        gains = np.concatenate(
            [np.asarray(q_gain[g * H_LOC:(g + 1) * H_LOC], dtype=np.float32),
             np.ones(KV_LOC, dtype=np.float32)])
        sc = 1.0 / (128.0 * gains * gains)
        bi = EPS / (gains * gains)
        qwt = permute_cols(
            np.ascontiguousarray(q_w[g * 1024:(g + 1) * 1024, :].T), H_LOC)
        kwt = permute_cols(
            np.ascontiguousarray(k_w[g * 256:(g + 1) * 256, :].T), KV_LOC)
        # single blob in the exact order build_program's `sizes` declares
        blob = np.concatenate([
            np.ascontiguousarray(x[b].T).astype(bf16).ravel(),
            qwt.astype(bf16).ravel(),
            kwt.astype(bf16).ravel(),
            np.ascontiguousarray(
                v_w[g * 256:(g + 1) * 256, :].T).astype(bf16).ravel(),
            np.ascontiguousarray(
                out_w[:, g * 1024:(g + 1) * 1024].T).astype(bf16).ravel(),
            cosp.ravel(),
            sinp.ravel(),
            masks.ravel(),
            np.concatenate([sc, bi]).astype(bf16),
        ])
        in_maps.append({"blob": blob})
    return in_maps


def kernel(x, q_w, k_w, v_w, out_w, q_gain, _trace=False, _trace_cores=None):
    x = np.asarray(x, dtype=np.float32)
    q_w = np.asarray(q_w, dtype=np.float32)
    k_w = np.asarray(k_w, dtype=np.float32)
    v_w = np.asarray(v_w, dtype=np.float32)
    out_w = np.asarray(out_w, dtype=np.float32)
    q_gain = np.asarray(q_gain, dtype=np.float32)
    T = x.shape[1]

    nc = build_program(T)
    in_maps = make_in_maps(x, q_w, k_w, v_w, out_w, q_gain, T)
    res = bass_utils.run_bass_kernel_spmd(
        nc, in_maps, core_ids=list(range(8)),
        trace=_trace, trace_cores=_trace_cores)
    outs = [r["out"] for r in res.results]
    full = np.stack([outs[2 * b] + outs[2 * b + 1] for b in range(B)])
    if _trace:
        return full.astype(np.float32), res
    return full.astype(np.float32)
